# revision 3
# baseline (speedup 1.0000x reference)
"""GAT (2-layer graph attention network) Bass kernel for 8 Trainium2 NeuronCores.

V2 design (trace-driven rewrite of the staged baseline):
- Edges partitioned by destination-node block; each core owns N/8 dst nodes
  (49 blocks of 128). Per-edge source rows come via SWDGE dma_gather.
- The baseline was bound by GPSIMD Q7 descriptor generation (~8.5ns/desc on
  ONE cpu pair, queue 0 only) and by HBM random-row rate. V2:
  * spreads gathers across the 4 SWDGE queues (4 Q7 cpu pairs in parallel),
  * eliminates the dst-alpha gathers entirely (half of all gathered rows):
    dst-alphas are broadcast to edge slots with small PE matmuls against
    host-precomputed transposed one-hot masks,
  * precomputes the {slot x dst} aggregation masks on the host (fp8) instead
    of DVE is_equal with degenerate broadcast access patterns,
  * head-interleaves layer-1 channels so DVE broadcasts have step-1 inner dims,
  * fuses the layer-2 input projection (phase C) into the per-block loop,
  * batches phase-A DMA into 4-tile groups.
- Global mean-pool partials are reduced on host; tiny fc head on host.
"""
import os
import sys
import types
import math

import numpy as np
import ml_dtypes


def _setup_paths():
    for p in ("/opt/trn_rl_repo", "/root/.axon_site/_ro/trn_rl_repo"):
        if os.path.isdir(p) and p not in sys.path:
            sys.path.insert(0, p)
    try:
        import concourse.bass  # noqa: F401
    except ImportError as e:
        raise RuntimeError(f"concourse not importable: {e}")


_setup_paths()

import concourse.bass as bass  # noqa: E402
import concourse.mybir as mybir  # noqa: E402
import concourse.tile as tile  # noqa: E402
from concourse import bacc, bass_utils  # noqa: E402

bf16 = ml_dtypes.bfloat16
fp8 = ml_dtypes.float8_e4m3fn
BF = mybir.dt.bfloat16
F32 = mybir.dt.float32
I16 = mybir.dt.int16
FP8 = mybir.dt.float8e4
AL = mybir.AluOpType
AF = mybir.ActivationFunctionType

MASK_DT = os.environ.get("KMASKDT", "fp8")


class Cfg:
    def __init__(self, N=50000, E=800000, IN_C=128, HID=64, OUT_C=64, HEADS=4,
                 NCLS=40, NEG=0.2, NCORES=8):
        self.N, self.E = N, E
        self.IN_C, self.HID, self.OUT_C, self.HEADS = IN_C, HID, OUT_C, HEADS
        self.NCLS, self.NEG, self.NCORES = NCLS, NEG, NCORES
        assert N % NCORES == 0
        self.NB = N // NCORES                      # owned real nodes per core
        self.NBLK = math.ceil(self.NB / 128)       # dst blocks per core
        self.NDP = self.NBLK * 128                 # padded owned rows per core
        self.RTOT = self.NDP * NCORES              # global padded row space
        self.NT1 = self.RTOT // 128                # phase-A node tiles
        # chunk-major row space: chunk0 = first 25 blocks of every core,
        # chunk1 = last 24 blocks; allows a 2-chunk allgather with chunk 0
        # overlapping the second half of phase B.
        self.CH0B = (self.NBLK + 1) // 2           # 25 blocks
        self.CH1B = self.NBLK - self.CH0B          # 24 blocks
        self.CH0R = self.CH0B * 128                # 3200 rows/core
        self.CH1R = self.CH1B * 128                # 3072 rows/core
        self.SPLIT = self.CH0R * NCORES            # 25600 (chunk0 size)
        assert self.SPLIT % 512 == 0               # 4-tile batch alignment
        assert self.SPLIT < 32768 and self.RTOT - self.SPLIT < 32768
        self.C1 = HEADS * HID                      # 256 layer-1 channels
        self.ROW1 = 384                            # t1 row cols (768B)
        self.ROW2 = 128                            # cc3 row cols (256B)

    def row_of(self, v):
        ci = v // self.NB
        rr = v % self.NB
        return np.where(
            rr < self.CH0R,
            ci * self.CH0R + rr,
            self.SPLIT + ci * self.CH1R + (rr - self.CH0R))


def _pack_idx(vals_2d):
    """vals_2d [G, n] -> dma_gather index layout [G, 128, n//16] int16.

    Index i lives at [i % 16, i // 16]; the 16-row group is replicated 8x
    across the 128 partitions.
    """
    G, n = vals_2d.shape
    assert n % 16 == 0
    a = vals_2d.reshape(G, n // 16, 16).transpose(0, 2, 1)   # [G, 16, n/16]
    return np.tile(a, (1, 8, 1)).astype(np.int16)            # [G, 128, n/16]


def host_prep(cfg, x, edge_index, W1, att_src1, att_dst1, b1, W2, att_src2,
              att_dst2, b2):
    c = cfg
    H = c.HEADS
    src = np.asarray(edge_index[0], dtype=np.int64)
    dst = np.asarray(edge_index[1], dtype=np.int64)
    loops = np.arange(c.N, dtype=np.int64)
    src = np.concatenate([src, loops])
    dst = np.concatenate([dst, loops])
    EE = src.shape[0]

    core = dst // c.NB
    drel = dst % c.NB                              # dst row within core
    blk = drel // 128
    din = drel % 128                               # dst index within block
    srow = c.row_of(src)
    isB = (srow >= c.SPLIT).astype(np.int64)

    gid = (core * c.NBLK + blk) * 2 + isB          # group id (A/B separate)
    order = np.argsort(gid, kind="stable")
    gid_s = gid[order]
    counts = np.bincount(gid_s, minlength=c.NCORES * c.NBLK * 2)
    nA = counts[0::2].reshape(c.NCORES, c.NBLK)
    nB = counts[1::2].reshape(c.NCORES, c.NBLK)
    kA = max(2, int(math.ceil(nA.max() / 128)))
    kA += kA % 2                                   # even: split across 2 queues
    kB = max(2, int(math.ceil(nB.max() / 128)))
    kB += kB % 2                                   # even: split across 2 queues
    K = kA + kB

    starts = np.zeros_like(counts)
    starts[1:] = np.cumsum(counts)[:-1]
    rank = np.arange(EE) - starts[gid_s]           # rank within (cg, A/B)

    cg = (core * c.NBLK + blk)[order]              # [EE] group (core, blk)
    rnk = rank.copy()                              # aligned with `order`
    isB_o = isB[order]
    srow_o = srow[order]
    din_o = din[order]

    NG = c.NCORES * c.NBLK
    idxA = np.zeros((NG, kA * 128), np.int64)
    idxB = np.zeros((NG, kB * 128), np.int64)
    selA = isB_o == 0
    selB = ~selA
    idxA[cg[selA], rnk[selA]] = srow_o[selA]
    idxB[cg[selB], rnk[selB]] = srow_o[selB] - c.SPLIT

    # trailing -1 truncation: descriptors are skipped for trailing negative
    # indices. Per (block, gather segment) the register count must be a
    # global (cross-core) literal, so pad with safe 0-idx up to the max real
    # count, -1 beyond it.
    kA2h, kB2h = kA // 2, kB // 2
    capA1, capB1 = kA2h * 128, kB2h * 128
    sA1 = np.minimum(nA, capA1); sA2 = nA - sA1        # [NCORES, NBLK]
    sB1 = np.minimum(nB, capB1); sB2 = nB - sB1
    regA1 = np.maximum(sA1.max(axis=0), 16)
    regA2 = np.maximum(sA2.max(axis=0), 16)
    regB1 = np.maximum(sB1.max(axis=0), 16)
    regB2 = np.maximum(sB2.max(axis=0), 16)
    iA3 = idxA.reshape(c.NCORES, c.NBLK, kA * 128)
    iB3 = idxB.reshape(c.NCORES, c.NBLK, kB * 128)
    colA = np.arange(kA * 128)[None, :]
    colB = np.arange(kB * 128)[None, :]
    mA = (colA >= regA1[:, None]) & (colA < capA1)
    mA |= colA >= capA1 + regA2[:, None]
    mB = (colB >= regB1[:, None]) & (colB < capB1)
    mB |= colB >= capB1 + regB2[:, None]
    iA3[:, mA] = -1
    iB3[:, mB] = -1

    # one-hot masks; slot s -> (j=s//128, p=s%128); B slots offset by kA*128
    mdt = fp8 if MASK_DT == "fp8" else bf16
    slot = np.where(selA, rnk, kA * 128 + rnk)
    mask = np.zeros((NG, K * 128, 128), mdt)       # [g, slot, dst]
    mask[cg, slot, din_o] = 1.0
    maskT = np.zeros((NG, 128, K * 128), mdt)      # [g, dst, slot]
    maskT[cg, din_o, slot] = 1.0
    # mask -> [g, 128 part, K, 128 dst]
    mask = mask.reshape(NG, K, 128, 128).transpose(0, 2, 1, 3)
    mask = np.ascontiguousarray(mask.reshape(c.NCORES, c.NBLK, 128, K * 128))
    maskT = np.ascontiguousarray(maskT.reshape(c.NCORES, c.NBLK, 128, K * 128))

    # ---- weights: head-interleave layer-1 channels (col = q*H + h) ----
    W1 = np.asarray(W1, np.float32)                # [128, H*HID]
    a_s1 = np.asarray(att_src1, np.float32).reshape(H, c.HID)
    a_d1 = np.asarray(att_dst1, np.float32).reshape(H, c.HID)
    W1r = W1.reshape(c.IN_C, H, c.HID)
    Wa_s = np.einsum("khc,hc->kh", W1r, a_s1)      # [IN_C, H]
    Wa_d = np.einsum("khc,hc->kh", W1r, a_d1)
    W1I = W1r.transpose(0, 2, 1).reshape(c.IN_C, c.C1)   # col = q*H + h
    w1aug = np.zeros((c.IN_C, c.C1 + 8), np.float32)
    w1aug[:, :c.C1] = W1I
    w1aug[:, c.C1:c.C1 + H] = Wa_s
    w1aug[:, c.C1 + 4:c.C1 + 4 + H] = Wa_d

    W2 = np.asarray(W2, np.float32)                # [256, 64]
    a_s2 = np.asarray(att_src2, np.float32).reshape(c.OUT_C)
    a_d2 = np.asarray(att_dst2, np.float32).reshape(c.OUT_C)
    W2I = W2.reshape(H, c.HID, c.OUT_C).transpose(1, 0, 2).reshape(c.C1, c.OUT_C)
    w2aug = np.zeros((c.C1, 72), np.float32)
    w2aug[:, :c.OUT_C] = W2I
    w2aug[:, c.OUT_C] = W2I @ a_s2
    w2aug[:, c.OUT_C + 1] = W2I @ a_d2

    assert np.allclose(np.asarray(b1), 0) and np.allclose(np.asarray(b2), 0), \
        "nonzero biases not folded in this build"

    # padded, row-mapped, transposed x tiles
    x = np.asarray(x, np.float32)
    x_pad = np.zeros((c.RTOT, c.IN_C), np.float32)
    rows = c.row_of(np.arange(c.N))
    x_pad[rows] = x
    xT = x_pad.reshape(c.NT1, 128, c.IN_C).transpose(0, 2, 1)  # [t, k, n]
    xT = np.ascontiguousarray(xT).astype(bf16)
    # per-core own tiles in block order (25 chunk0 tiles + 24 chunk1 tiles)
    x_owns = []
    for ci in range(c.NCORES):
        t0 = ci * c.CH0B
        t1 = c.SPLIT // 128 + ci * c.CH1B
        x_owns.append(np.ascontiguousarray(np.concatenate(
            [xT[t0:t0 + c.CH0B], xT[t1:t1 + c.CH1B]], axis=0)))

    ones = np.ones((128, 1), np.float32)

    in_maps = []
    meta = dict(kA=kA, kB=kB, K=K,
                regs=(tuple(int(x) for x in regA1),
                      tuple(int(x) for x in regA2),
                      tuple(int(x) for x in regB1),
                      tuple(int(x) for x in regB2)))
    for ci in range(c.NCORES):
        s = slice(ci * c.NBLK, (ci + 1) * c.NBLK)
        in_maps.append({
            "x_t3": xT,
            "x_own": x_owns[ci],
            "w1aug": w1aug.astype(bf16),
            "w2aug": np.ascontiguousarray(
                w2aug.astype(bf16).reshape(2, 128, 72).transpose(1, 0, 2)),
            "idxA": np.ascontiguousarray(_pack_idx(idxA[s]).transpose(1, 0, 2)),
            "idxB": np.ascontiguousarray(_pack_idx(idxB[s]).transpose(1, 0, 2)),
            "mask": mask[ci],
            "maskT": maskT[ci],
            "ones": ones,
        })
    return in_maps, meta


def build(cfg, kA, kB, regs=None, stage="F"):
    c = cfg
    K = kA + kB
    kA2 = kA // 2
    kB2 = kB // 2
    if regs is None:
        regs = (tuple([kA2 * 128] * c.NBLK), tuple([kA2 * 128] * c.NBLK),
                tuple([kB2 * 128] * c.NBLK), tuple([kB2 * 128] * c.NBLK))
    regA1, regA2, regB1, regB2 = regs
    MD = FP8 if MASK_DT == "fp8" else BF
    nc = bacc.Bacc("TRN2", target_bir_lowering=False, debug=False,
                   num_devices=c.NCORES, num_swdge_queues=4)

    # ---- IO ----
    x_t3_d = nc.dram_tensor("x_t3", [c.NT1, 128, c.IN_C], BF, kind="ExternalInput").ap()
    x_own_d = nc.dram_tensor("x_own", [c.NBLK, 128, c.IN_C], BF, kind="ExternalInput").ap()
    w1_d = nc.dram_tensor("w1aug", [c.IN_C, c.C1 + 8], BF, kind="ExternalInput").ap()
    w2_d = nc.dram_tensor("w2aug", [128, 2, 72], BF, kind="ExternalInput").ap()
    idxA_d = nc.dram_tensor("idxA", [128, c.NBLK, kA * 8], I16, kind="ExternalInput").ap()
    idxB_d = nc.dram_tensor("idxB", [128, c.NBLK, kB * 8], I16, kind="ExternalInput").ap()
    mask_d = nc.dram_tensor("mask", [c.NBLK, 128, K * 128], MD, kind="ExternalInput").ap()
    maskT_d = nc.dram_tensor("maskT", [c.NBLK, 128, K * 128], MD, kind="ExternalInput").ap()
    ones_d = nc.dram_tensor("ones", [128, 1], F32, kind="ExternalInput").ap()
    pool_d = nc.dram_tensor("pool64", [c.OUT_C, 1], F32, kind="ExternalOutput").ap()

    # ---- internal DRAM ----
    t1A = nc.dram_tensor("t1A", [c.SPLIT, c.ROW1], BF, kind="Internal").ap()
    t1B = nc.dram_tensor("t1B", [c.RTOT - c.SPLIT, c.ROW1], BF, kind="Internal").ap()
    cc3inA = nc.dram_tensor("cc3inA", [c.CH0R, c.ROW2], BF, kind="Internal").ap()
    cc3inB = nc.dram_tensor("cc3inB", [c.CH1R, c.ROW2], BF, kind="Internal").ap()
    t3A = nc.dram_tensor("t3A", [c.SPLIT, c.ROW2], BF, kind="Internal",
                         addr_space="Shared").ap()
    t3B = nc.dram_tensor("t3B", [c.RTOT - c.SPLIT, c.ROW2], BF, kind="Internal",
                         addr_space="Shared").ap()

    nbatch = c.NT1 // 4                    # 98 groups of 4 tiles
    split_g = c.SPLIT // 512               # t1A/t1B boundary in 4-tile groups

    with tile.TileContext(nc) as tc:
        with tc.tile_pool(name="const", bufs=1) as cpool, \
             tc.tile_pool(name="pa", bufs=3) as pa, \
             tc.tile_pool(name="pp", bufs=2, space="PSUM") as pp, \
             tc.tile_pool(name="pp1", bufs=1, space="PSUM") as pp1, \
             tc.tile_pool(name="pg", bufs=2) as pg, \
             tc.tile_pool(name="pe2", bufs=2) as pe2, \
             tc.tile_pool(name="sm", bufs=3) as sm:

            w1s = cpool.tile_from(w1_d)                     # [128, 264]
            w2s = cpool.tile_from(w2_d)                     # [128, 2, 72]
            idxA_s = cpool.tile_from(idxA_d)
            idxB_s = cpool.tile_from(idxB_d)
            ones_s = cpool.tile_from(ones_d)
            pacc = cpool.tile([128, c.OUT_C], F32)
            nc.vector.memset(pacc[:], 0.0)
            adown_s = cpool.tile([128, c.NBLK, 4], BF)      # dst-alphas L1
            a2_s = cpool.tile([128, c.NBLK, 1], BF)         # dst-alphas L2

            # ================= phase A: h table (replicated) =================
            for g in range(nbatch):
                xt4 = pa.tile([128, 4, c.IN_C], BF, tag="xt")
                nc.scalar.dma_start(
                    out=xt4[:],
                    in_=x_t3_d[g * 4:(g + 1) * 4, :, :].rearrange("t p c -> p t c"))
                hb4 = pa.tile([128, 4, c.C1 + 8], BF, tag="hb")
                for i in range(4):
                    ps = pp.tile([128, c.C1 + 8], F32, tag="AB")
                    nc.tensor.matmul(out=ps[:], lhsT=xt4[:, i, :], rhs=w1s[:],
                                     start=True, stop=True)
                    if i % 2 == 0:
                        nc.scalar.activation(out=hb4[:, i, :], in_=ps[:],
                                             func=AF.Copy)
                    else:
                        nc.vector.tensor_copy(out=hb4[:, i, :], in_=ps[:])
                r0 = g * 512
                if g < split_g:
                    dst_ap = t1A[r0:r0 + 512, 0:c.C1 + 8]
                else:
                    dst_ap = t1B[r0 - c.SPLIT:r0 - c.SPLIT + 512, 0:c.C1 + 8]
                nc.sync.dma_start(
                    out=dst_ap.rearrange("(t p) c -> p t c", p=128), in_=hb4[:])

            # own-rows dst-alpha table (SBUF-resident)
            for j in range(c.NBLK):
                xo = pa.tile([128, c.IN_C], BF, tag="xo")
                nc.sync.dma_start(out=xo[:], in_=x_own_d[j, :, :])
                pso = pp.tile([128, c.C1 + 8], F32, tag="AB")
                nc.tensor.matmul(out=pso[:, 0:8], lhsT=xo[:],
                                 rhs=w1s[:, c.C1:c.C1 + 8], start=True, stop=True)
                nc.scalar.activation(out=adown_s[:, j, :], in_=pso[:, 4:8],
                                     func=AF.Copy)

            # ============== phase B+C: layer-1 aggregation + proj ============
            if stage >= "B":
                for _t in range(2):   # zero gather buffers once (NaN safety:
                    hgz = pg.tile([128, K, c.ROW1], BF, tag="hg")
                    nc.vector.memset(hgz[:], 0.0)   # truncated slots stay stale)
            for b in (range(c.NBLK) if stage >= "B" else []):
                hg = pg.tile([128, K, c.ROW1], BF, tag="hg")
                nc.gpsimd.dma_gather(
                    out_ap=hg[:, 0:kA2, :], in_ap=t1A[:, :],
                    idxs_ap=idxA_s[:, b, 0:kA2 * 8], num_idxs=kA2 * 128,
                    num_idxs_reg=regA1[b], elem_size=c.ROW1,
                    single_packet=False, queue_num=0)
                nc.gpsimd.dma_gather(
                    out_ap=hg[:, kA2:kA, :], in_ap=t1A[:, :],
                    idxs_ap=idxA_s[:, b, kA2 * 8:kA * 8], num_idxs=kA2 * 128,
                    num_idxs_reg=regA2[b], elem_size=c.ROW1,
                    single_packet=False, queue_num=2)
                nc.gpsimd.dma_gather(
                    out_ap=hg[:, kA:kA + kB2, :], in_ap=t1B[:, :],
                    idxs_ap=idxB_s[:, b, 0:kB2 * 8], num_idxs=kB2 * 128,
                    num_idxs_reg=regB1[b], elem_size=c.ROW1,
                    single_packet=False, queue_num=1)
                nc.gpsimd.dma_gather(
                    out_ap=hg[:, kA + kB2:K, :], in_ap=t1B[:, :],
                    idxs_ap=idxB_s[:, b, kB2 * 8:kB * 8], num_idxs=kB2 * 128,
                    num_idxs_reg=regB2[b], elem_size=c.ROW1,
                    single_packet=False, queue_num=3)
                mk = pg.tile([128, K, 128], MD, tag="mk")
                nc.sync.dma_start(out=mk[:], in_=mask_d[b, :, :])
                mkT = pg.tile([128, K, 128], MD, tag="mkT")
                nc.sync.dma_start(out=mkT[:], in_=maskT_d[b, :, :])

                # dst-alpha broadcast: ad_slot[p,j,:] = adown[dst(p,j),:]
                adps = pp1.tile([128, K, 4], F32, tag="adps")
                for j in range(K):
                    nc.tensor.matmul(out=adps[:, j, :], lhsT=mkT[:, j, :],
                                     rhs=adown_s[:, b, :], start=True, stop=True)
                adsl = sm.tile([128, K, 4], F32, tag="adsl")
                nc.vector.tensor_copy(out=adsl[:], in_=adps[:])

                # segment-split compute: A-half work starts as soon as the
                # t1A gathers land, overlapping the t1B gather DMA.
                z = sm.tile([128, K, 4], F32, tag="z")
                lr = sm.tile([128, K, 4], F32, tag="lr")
                v = pg.tile([128, K, c.C1 + 8], BF, tag="v")
                ps = pp.tile([128, c.C1 + 8], F32, tag="AB")
                for (j0, j1) in ((0, kA), (kA, K)):
                    kk = j1 - j0
                    nc.vector.tensor_tensor(
                        out=z[:, j0:j1, :], in0=hg[:, j0:j1, c.C1:c.C1 + 4],
                        in1=adsl[:, j0:j1, :], op=AL.add)
                    nc.vector.scalar_tensor_tensor(
                        out=lr[:, j0:j1, :], in0=z[:, j0:j1, :], scalar=c.NEG,
                        in1=z[:, j0:j1, :], op0=AL.mult, op1=AL.max)
                    nc.scalar.activation(out=v[:, j0:j1, c.C1:c.C1 + 4],
                                         in_=lr[:, j0:j1, :], func=AF.Exp)
                    nc.vector.tensor_tensor(
                        out=v[:, j0:j1, 0:c.C1].rearrange(
                            "p k (q h) -> p k q h", h=4),
                        in0=hg[:, j0:j1, 0:c.C1].rearrange(
                            "p k (q h) -> p k q h", h=4),
                        in1=v[:, j0:j1, c.C1:c.C1 + 4][:, :, None, :]
                            .to_broadcast([128, kk, c.HID, 4]),
                        op=AL.mult)
                    for j in range(j0, j1):
                        nc.tensor.matmul(out=ps[:, 0:c.C1 + 4],
                                         lhsT=mk[:, j, :],
                                         rhs=v[:, j, 0:c.C1 + 4],
                                         start=(j == 0), stop=(j == K - 1))

                den = sm.tile([128, 4], F32, tag="den")
                nc.vector.tensor_scalar(
                    out=den[:], in0=ps[:, c.C1:c.C1 + 4], scalar1=1e-16,
                    scalar2=None, op0=AL.add)
                rec = sm.tile([128, 4], F32, tag="rec")
                nc.vector.reciprocal(out=rec[:], in_=den[:])
                h1f = sm.tile([128, c.C1], F32, tag="h1f")
                nc.vector.tensor_tensor(
                    out=h1f[:].rearrange("p (q h) -> p q h", h=4),
                    in0=ps[:, 0:c.C1].rearrange("p (q h) -> p q h", h=4),
                    in1=rec[:, None, :].to_broadcast([128, c.HID, 4]),
                    op=AL.mult)
                h1b = sm.tile([128, c.C1], BF, tag="h1b")
                nc.vector.tensor_scalar(
                    out=h1b[:], in0=h1f[:], scalar1=0.0, scalar2=None,
                    op0=AL.max)
                # --- fused phase C: h2pre = relu(h1) @ W2aug ---
                ht = sm.tile([128, 2, 128], BF, tag="ht")
                for kh in range(2):
                    nc.sync.dma_start(
                        out=ht[:, kh, :],
                        in_=h1b[:, kh * 128:(kh + 1) * 128],
                        transpose=True)
                psc = pp1.tile([128, 72], F32, tag="C")
                for kh in range(2):
                    nc.tensor.matmul(out=psc[:], lhsT=ht[:, kh, :],
                                     rhs=w2s[:, kh, :],
                                     start=(kh == 0), stop=(kh == 1))
                hc = sm.tile([128, 72], BF, tag="hc")
                nc.vector.tensor_copy(out=hc[:], in_=psc[:])
                nc.vector.tensor_copy(out=a2_s[:, b, :], in_=psc[:, 65:66])
                if b < c.CH0B:
                    nc.sync.dma_start(
                        out=cc3inA[b * 128:(b + 1) * 128, 0:72], in_=hc[:])
                else:
                    bb = b - c.CH0B
                    nc.sync.dma_start(
                        out=cc3inB[bb * 128:(bb + 1) * 128, 0:72], in_=hc[:])
                if b == c.CH0B - 1 and stage >= "D":
                    nc.gpsimd.collective_compute(
                        kind="AllGather", op=AL.bypass,
                        replica_groups=[list(range(c.NCORES))],
                        ins=[cc3inA[:, :]], outs=[t3A[:, :]])

            # ================= phase D: allgather (chunk 1) ==================
            if stage >= "D":
                nc.gpsimd.collective_compute(
                    kind="AllGather", op=AL.bypass,
                    replica_groups=[list(range(c.NCORES))],
                    ins=[cc3inB[:, :]], outs=[t3B[:, :]])

            # ================= phase E: layer-2 aggregation ==================
            if stage >= "E":
                for _t in range(2):
                    hgz2 = pe2.tile([128, K, c.ROW2], BF, tag="hg2")
                    nc.vector.memset(hgz2[:], 0.0)
            for b in (range(c.NBLK) if stage >= "E" else []):
                hg2 = pe2.tile([128, K, c.ROW2], BF, tag="hg2")
                nc.gpsimd.dma_gather(
                    out_ap=hg2[:, 0:kA2, :], in_ap=t3A[:, :],
                    idxs_ap=idxA_s[:, b, 0:kA2 * 8], num_idxs=kA2 * 128,
                    num_idxs_reg=regA1[b], elem_size=c.ROW2,
                    single_packet=False, queue_num=0)
                nc.gpsimd.dma_gather(
                    out_ap=hg2[:, kA2:kA, :], in_ap=t3A[:, :],
                    idxs_ap=idxA_s[:, b, kA2 * 8:kA * 8], num_idxs=kA2 * 128,
                    num_idxs_reg=regA2[b], elem_size=c.ROW2,
                    single_packet=False, queue_num=2)
                nc.gpsimd.dma_gather(
                    out_ap=hg2[:, kA:kA + kB2, :], in_ap=t3B[:, :],
                    idxs_ap=idxB_s[:, b, 0:kB2 * 8], num_idxs=kB2 * 128,
                    num_idxs_reg=regB1[b], elem_size=c.ROW2,
                    single_packet=False, queue_num=1)
                nc.gpsimd.dma_gather(
                    out_ap=hg2[:, kA + kB2:K, :], in_ap=t3B[:, :],
                    idxs_ap=idxB_s[:, b, kB2 * 8:kB * 8], num_idxs=kB2 * 128,
                    num_idxs_reg=regB2[b], elem_size=c.ROW2,
                    single_packet=False, queue_num=3)
                mk2 = pe2.tile([128, K, 128], MD, tag="mk2")
                nc.sync.dma_start(out=mk2[:], in_=mask_d[b, :, :])
                mkT2 = pe2.tile([128, K, 128], MD, tag="mkT2")
                nc.sync.dma_start(out=mkT2[:], in_=maskT_d[b, :, :])

                a2ps = pp1.tile([128, K], F32, tag="a2ps")
                for j in range(K):
                    nc.tensor.matmul(out=a2ps[:, j:j + 1], lhsT=mkT2[:, j, :],
                                     rhs=a2_s[:, b, :], start=True, stop=True)
                ad2sl = sm.tile([128, K], F32, tag="ad2sl")
                nc.vector.tensor_copy(out=ad2sl[:], in_=a2ps[:])

                z2 = sm.tile([128, K, 1], F32, tag="z2")
                lr2 = sm.tile([128, K, 1], F32, tag="lr2")
                v2 = pe2.tile([128, K, c.OUT_C + 1], BF, tag="v2")
                ps2 = pp.tile([128, c.OUT_C + 1], F32, tag="E")
                for (j0, j1) in ((0, kA), (kA, K)):
                    kk = j1 - j0
                    nc.vector.tensor_tensor(
                        out=z2[:, j0:j1, :],
                        in0=hg2[:, j0:j1, c.OUT_C:c.OUT_C + 1],
                        in1=ad2sl[:, j0:j1, None], op=AL.add)
                    nc.vector.scalar_tensor_tensor(
                        out=lr2[:, j0:j1, :], in0=z2[:, j0:j1, :], scalar=c.NEG,
                        in1=z2[:, j0:j1, :], op0=AL.mult, op1=AL.max)
                    nc.scalar.activation(out=v2[:, j0:j1, c.OUT_C:c.OUT_C + 1],
                                         in_=lr2[:, j0:j1, :], func=AF.Exp)
                    nc.vector.tensor_tensor(
                        out=v2[:, j0:j1, 0:c.OUT_C], in0=hg2[:, j0:j1, 0:c.OUT_C],
                        in1=v2[:, j0:j1, c.OUT_C:c.OUT_C + 1].to_broadcast(
                            [128, kk, c.OUT_C]),
                        op=AL.mult)
                    for j in range(j0, j1):
                        nc.tensor.matmul(out=ps2[:], lhsT=mk2[:, j, :],
                                         rhs=v2[:, j, :],
                                         start=(j == 0), stop=(j == K - 1))

                den2 = sm.tile([128, 1], F32, tag="den2")
                nc.vector.tensor_scalar(
                    out=den2[:], in0=ps2[:, c.OUT_C:c.OUT_C + 1], scalar1=1e-16,
                    scalar2=None, op0=AL.add)
                rec2 = sm.tile([128, 1], F32, tag="rec2")
                nc.vector.reciprocal(out=rec2[:], in_=den2[:])
                o2 = sm.tile([128, c.OUT_C], F32, tag="o2")
                nc.vector.tensor_tensor(
                    out=o2[:], in0=ps2[:, 0:c.OUT_C],
                    in1=rec2[:, 0:1].to_broadcast([128, c.OUT_C]), op=AL.mult)
                o2r = sm.tile([128, c.OUT_C], F32, tag="o2r")
                nc.vector.tensor_scalar(
                    out=o2r[:], in0=o2[:], scalar1=0.0, scalar2=None,
                    op0=AL.max)
                nc.vector.tensor_tensor(out=pacc[:], in0=pacc[:], in1=o2r[:],
                                        op=AL.add)

            # ================= phase F: pool partial =========================
            psf = pp.tile([128, c.OUT_C + 1], F32, tag="E")
            nc.tensor.matmul(out=psf[0:c.OUT_C, 0:1], lhsT=pacc[:], rhs=ones_s[:],
                             start=True, stop=True)
            pf = sm.tile([c.OUT_C, 1], F32, tag="pf")
            nc.vector.tensor_copy(out=pf[:], in_=psf[0:c.OUT_C, 0:1])
            nc.sync.dma_start(out=pool_d[:, :], in_=pf[:])

    nc.compile()
    legalize_waits(nc)
    return nc


def legalize_waits(nc):
    """Walrus encodes at most ONE sync wait per instruction on this toolchain.
    Hoist excess waits onto same-engine NoOps inserted before the instruction."""
    for fn in nc.m.functions:
        for bb in fn.blocks:
            insts = list(bb.instructions)
            out = []
            changed = False
            for inst in insts:
                si = inst.sync_info
                if si is not None and si.on_wait and len(si.on_wait) > 1:
                    waits = list(si.on_wait)
                    for w in waits[:-1]:
                        nop = mybir.InstNoOp(
                            name=nc.get_next_instruction_name(), ins=[], outs=[])
                        nop.engine = inst.engine
                        nop.sync_info = mybir.SyncInfo(on_wait=[w], on_update=[])
                        nc.register_instruction(nop)
                        out.append(nop)
                    inst.sync_info = mybir.SyncInfo(
                        on_wait=waits[-1:], on_update=list(si.on_update))
                    changed = True
                out.append(inst)
            if changed:
                bb.instructions.clear()
                bb.instructions.extend(out)


def host_finish(cfg, pools, fc_w, fc_b):
    c = cfg
    tot = np.zeros(c.OUT_C, np.float64)
    for p in pools:
        tot += p[:, 0].astype(np.float64)
    pooled = (tot / c.N).astype(np.float32)
    logits = pooled @ np.asarray(fc_w, np.float32) + np.asarray(fc_b, np.float32)
    m = logits.max()
    ls = logits - (m + np.log(np.exp(logits - m).sum()))
    return ls.reshape(1, c.NCLS).astype(np.float32)


_BUILD_CACHE = {}


def run(cfg, inputs, trace=False, **run_kwargs):
    in_maps, meta = host_prep(
        cfg, inputs["x"], inputs["edge_index"], inputs["W1"], inputs["att_src1"],
        inputs["att_dst1"], inputs["b1"], inputs["W2"], inputs["att_src2"],
        inputs["att_dst2"], inputs["b2"])
    stage = os.environ.get("KSTAGE", "F")
    key = (cfg.N, cfg.E, meta["kA"], meta["kB"], meta["regs"], stage)
    if key not in _BUILD_CACHE:
        _BUILD_CACHE[key] = build(cfg, meta["kA"], meta["kB"],
                                  regs=meta["regs"], stage=stage)
    nc = _BUILD_CACHE[key]
    res = bass_utils.run_bass_kernel_spmd(
        nc, in_maps, core_ids=list(range(cfg.NCORES)), trace=trace, **run_kwargs)
    out = host_finish(cfg, [r["pool64"] for r in res.results],
                      inputs["fc_w"], inputs["fc_b"])
    return out, res


def kernel(**inputs):
    cfg = Cfg()
    out, _ = run(cfg, inputs)
    return out
